# revision 12
# baseline (speedup 1.0000x reference)
"""MoE top-1 routing kernel for Trainium2 (8 NeuronCores).

Reference computation (B=8, S=1024, D=768, E=8, F=3072):
    gates = softmax(x @ gate_w + gate_b); expert_idx = argmax(gates)
    out[t] = gelu(x[t] @ w1[e] + b1[e]) @ w2[e] + b2[e]   for e = expert_idx[t]
    (no gate-probability scaling)

Strategy (v2 — zero-padding feature-parallel):
  * Routing on host in fp64 (softmax is monotonic, so argmax of logits ==
    argmax of gates; observed top-2 logit gaps are >=2e-5, far above fp32
    matmul noise, so this matches the reference's argmax).
  * Every core processes ALL 8192 tokens (sorted by expert) but only a
    1/8 slice of the F dimension of every expert's weights (384 features).
    Core q holds w1[:, q*384:(q+1)*384] and w2[q*384:(q+1)*384, :] for all
    8 experts (9.4 MB bf16) and produces a partial sum of the second
    matmul; the host adds the 8 partials + b2.  This gives exactly T/8
    worth of MACs per core with ZERO padding, for any routing balance —
    vs ~100-token padding for the grouped expert-parallel split.
  * Matmuls in bf16 with fp32 PSUM accumulation; activations stay
    transposed ([feature, token]) so both weight matrices act as the
    stationary matmul operand in their natural layout.  gelu (erf-based)
    on the Scalar engine with the b1 bias fused; FFN2 partial-sums are
    copied PSUM->SBUF as bf16 on the Vector engine and DMA'd out.
  * Head: DMAs are issued in exact consumption order (b1, first expert's
    first w1 chunk, a small 128-token lead tile, ...), batched one-per-
    object so issue cost doesn't serialize; a short PE warmup spins while
    the first ~400 KB lands, then real matmuls start (possibly still at
    the cold 1.2 GHz clock — still useful work) and the HAM clock-gate
    lifts to 2.4 GHz.  Tail: the last tile is kept small and its output
    DMA is split per-chunk so the final write receipt trails as little
    compute as possible.
"""

import sys

try:
    import concourse  # noqa: F401
except ImportError:
    sys.path.insert(0, "/opt/trn_rl_repo")

import numpy as np
import ml_dtypes

import concourse.bass as bass  # noqa: F401
import concourse.tile as tile
import concourse.mybir as mybir
from concourse import bacc
from concourse import bass_utils

BF16 = mybir.dt.bfloat16
F32 = mybir.dt.float32
AF = mybir.ActivationFunctionType

B, S, D, E = 8, 1024, 768, 8
F = 4 * D           # 3072
T = B * S           # 8192
KD = D // 128       # 6 contraction chunks over D
NQ = 8              # F-slice factor (all 8 cores)
FQ = F // NQ        # 384 features per core
KQ = FQ // 128      # 3 chunks over the F-slice
N_CORES = 8
MAX_N = 512         # moving-dim tile (one fp32 PSUM bank)
LEAD = 128          # first tile size (minimal first-matmul DMA dependency)
TAIL = 128          # last tile size (short output-DMA tail)
TAIL2 = 192         # second-to-last tile size
N_WARM = 22         # PE warmup matmuls (~2.4us cold) while head DMAs land

# Debug/profiling knobs (used by the local test harness only).
TRACE = False
LAST_RESULT = None


def _split_tiles(cap):
    """Split a block of `cap` tokens into ceil(cap/512) near-equal tiles."""
    if cap == 0:
        return []
    n = -(-cap // MAX_N)
    base, rem = divmod(cap, n)
    out = []
    off = 0
    for i in range(n):
        sz = base + (1 if i < rem else 0)
        out.append((off, sz))
        off += sz
    return out


def _schedule(counts):
    """(block, tile-offset, width) list over 8 expert blocks, descending
    count order.  First tile is LEAD-sized; last tile is <= TAIL."""
    order = sorted(range(E), key=lambda e: -counts[e])
    offs = np.zeros(E + 1, dtype=int)
    for i, e in enumerate(order):
        offs[i + 1] = offs[i] + counts[e]
    sched = []
    for i, e in enumerate(order):
        tiles = _split_tiles(counts[e])
        if i == 0 and tiles and tiles[0][1] > LEAD + 64:
            o, w = tiles[0]
            tiles = [(o, LEAD), (o + LEAD, w - LEAD)] + tiles[1:]
        if i == E - 1 and tiles and tiles[-1][1] > TAIL + TAIL2 + 64:
            o, w = tiles[-1]
            tiles = tiles[:-1] + [(o, w - TAIL - TAIL2),
                                  (o + w - TAIL - TAIL2, TAIL2),
                                  (o + w - TAIL, TAIL)]
        for (o, w) in tiles:
            sched.append((i, offs[i] + o, w))
    return order, offs, sched


def build_program(counts):
    """Per-core program: 8 expert blocks (descending size) over the full
    sorted token stream; each core computes a 1/8-F partial of FFN1+FFN2."""
    counts = list(counts)
    order, offs, sched = _schedule(counts)
    nc = bacc.Bacc("TRN2", target_bir_lowering=False, debug=False,
                   num_devices=N_CORES)

    xT_d = nc.dram_tensor("xT", (128, KD, T), BF16, kind="ExternalInput")
    w1_d = nc.dram_tensor("w1", (128, E, KQ, KD, 128), BF16,
                          kind="ExternalInput")
    w2_d = nc.dram_tensor("w2", (128, E, KD, KQ, 128), BF16,
                          kind="ExternalInput")
    b1_d = nc.dram_tensor("b1", (128, E, KQ), F32, kind="ExternalInput")
    yT_d = nc.dram_tensor("yT", (128, KD, T), BF16, kind="ExternalOutput")

    with tile.TileContext(nc) as tc:
        with (
            tc.tile_pool(name="wts", bufs=1) as wts,
            tc.tile_pool(name="act", bufs=2) as actp,
            tc.tile_pool(name="yp", bufs=3) as yp,
            tc.tile_pool(name="ps1", bufs=3, space="PSUM") as ps1,
            tc.tile_pool(name="ps2", bufs=5, space="PSUM") as ps2,
        ):
            xT = wts.tile([128, KD, T], BF16, tag="xT")
            w1 = wts.tile([128, E, KQ, KD, 128], BF16, tag="w1")
            w2 = wts.tile([128, E, KD, KQ, 128], BF16, tag="w2")
            b1 = wts.tile([128, E, KQ], F32, tag="b1")
            warm = wts.tile([128, 128], BF16, tag="warm")
            nc.gpsimd.memset(warm[:], 0.0)
            wps = ps2.tile([128, 128], F32, tag="ps2",
                           padded_shape=[128, MAX_N])

            # PE warmup: dummy matmuls keep the PE busy while the head DMAs
            # stream in, flipping the HAM clock gate toward 2.4 GHz before
            # (or shortly after) the real matmul stream starts.
            for _ in range(N_WARM):
                nc.tensor.matmul(wps[:, :], warm[:, :], warm[:, :])

            # DMA plan.  Everything except two tiny head transfers rides
            # the Sync HWDGE queue (a single FIFO gives full control of
            # delivery order); instructions are interleaved with the tile
            # emission so inputs arrive exactly one block ahead of use and
            # output tiles never sit behind a deep input backlog.  The
            # Scalar engine (gelu stream) only issues b1 + the very first
            # w1 chunk up front, and the last tile's output at the end.
            blocks = [i for i in range(E) if counts[order[i]] > 0]
            tiles_of = {i: [(n0, nt) for (bi, n0, nt) in sched if bi == i]
                        for i in blocks}
            e0 = order[blocks[0]]
            nc.scalar.dma_start(w1[:, e0, 0], w1_d[:, e0, 0])
            nc.scalar.dma_start(w1[:, e0, 1], w1_d[:, e0, 1])
            nc.scalar.dma_start(b1[:], b1_d[:])

            def dma_xt(n0, nt):
                nc.sync.dma_start(xT[:, :, n0:n0 + nt],
                                  xT_d[:, :, n0:n0 + nt])

            # Head: block 0 completely, plus the next block's w1.
            b0_tiles = tiles_of[blocks[0]]
            dma_xt(*b0_tiles[0])
            nc.sync.dma_start(w1[:, e0, 2], w1_d[:, e0, 2])
            if len(b0_tiles) > 1:
                dma_xt(*b0_tiles[1])
            nc.sync.dma_start(w2[:, e0], w2_d[:, e0])
            for (n0, nt) in b0_tiles[2:]:
                dma_xt(n0, nt)
            if len(blocks) > 1:
                e1 = order[blocks[1]]
                nc.sync.dma_start(w1[:, e1], w1_d[:, e1])

            # JIT plan: during block k, pull block k+1's w2 + tokens and
            # block k+2's w1, spread round-robin over block k's tiles.
            # jit[tile_global_index] = list of thunks to issue there.
            jit = [[] for _ in sched]
            tile_base = {}
            tb = 0
            for i in blocks:
                tile_base[i] = tb
                tb += len(tiles_of[i])
            for ki, i in enumerate(blocks):
                items = []
                if ki + 1 < len(blocks):
                    nxt = order[blocks[ki + 1]]
                    items.append(lambda e=nxt: nc.sync.dma_start(
                        w2[:, e], w2_d[:, e]))
                    for (n0, nt) in tiles_of[blocks[ki + 1]]:
                        items.append(lambda n0=n0, nt=nt: dma_xt(n0, nt))
                if ki + 2 < len(blocks):
                    nx2 = order[blocks[ki + 2]]
                    items.append(lambda e=nx2: nc.sync.dma_start(
                        w1[:, e], w1_d[:, e]))
                ntiles = len(tiles_of[i])
                for j, th in enumerate(items):
                    jit[tile_base[i] + min(j, ntiles - 1)].append(th)

            def ffn1(e, n0, nt):
                h = actp.tile([128, KQ, nt], BF16, tag="h",
                              padded_shape=[128, KQ, MAX_N])
                for m in range(KQ):
                    ps = ps1.tile([128, nt], F32, tag="ps1",
                                  padded_shape=[128, MAX_N])
                    for k in range(KD):
                        nc.tensor.matmul(
                            ps[:, :],
                            w1[:, e, m, k, :],
                            xT[:, k, n0:n0 + nt],
                            start=(k == 0),
                            stop=(k == KD - 1),
                        )
                    nc.scalar.activation(h[:, m, :], ps[:, :], AF.Gelu,
                                         bias=b1[:, e, m:m + 1])
                return h

            def ffn2(e, n0, nt, h, out_eng):
                y = yp.tile([128, KD, nt], BF16, tag="y",
                            padded_shape=[128, KD, MAX_N])
                for md in range(KD):
                    ps = ps2.tile([128, nt], F32, tag="ps2",
                                  padded_shape=[128, MAX_N])
                    for k in range(KQ):
                        nc.tensor.matmul(
                            ps[:, :],
                            w2[:, e, md, k, :],
                            h[:, k, :],
                            start=(k == 0),
                            stop=(k == KQ - 1),
                        )
                    nc.vector.tensor_copy(y[:, md, :], ps[:, :])
                out_eng.dma_start(yT_d[:, :, n0:n0 + nt], y[:, :, :])

            # Software-pipelined emission: FFN1(t) ahead of FFN2(t-1) so the
            # PE never waits on the gelu of the tile it just produced.  The
            # JIT input DMAs for block k+1 are issued at their scheduled
            # tiles; output DMAs follow each tile's casts on the same Sync
            # queue (the last tile's goes on the empty Scalar queue so its
            # write receipt isn't stuck behind a draining transfer).
            prev = None
            for ti, (b, n0, nt) in enumerate(sched):
                for th in jit[ti]:
                    th()
                h = ffn1(order[b], n0, nt)
                if prev is not None:
                    ffn2(*prev)
                out_eng = nc.scalar if ti >= len(sched) - 2 else nc.sync
                prev = (order[b], n0, nt, h, out_eng)
            if prev is not None:
                ffn2(*prev)

    nc.compile()
    return nc


_PROGRAM_CACHE = {}


def _get_program(counts):
    key = tuple(counts)
    if key not in _PROGRAM_CACHE:
        _PROGRAM_CACHE[key] = build_program(counts)
    return _PROGRAM_CACHE[key]


def kernel(x, gate_w, gate_b, w1, b1, w2, b2):
    x = np.asarray(x)
    w1 = np.asarray(w1)
    b1 = np.asarray(b1)
    w2 = np.asarray(w2)
    b2 = np.asarray(b2)
    xt = x.reshape(T, D)

    # --- Routing on host (fp64; softmax is monotonic => argmax of logits) ---
    logits = xt.astype(np.float64) @ np.asarray(gate_w, np.float64)
    logits += np.asarray(gate_b, np.float64)
    eidx = np.argmax(logits, axis=-1)
    counts = np.bincount(eidx, minlength=E)

    nc = _get_program(counts)
    order, offs, _ = _schedule(counts)

    # Token stream sorted by expert in block order (descending count).
    idx_blocks = [np.nonzero(eidx == e)[0] for e in order]
    sort_idx = np.concatenate(idx_blocks)
    xs = xt[sort_idx].astype(ml_dtypes.bfloat16)        # [T, D]
    # [T, D] -> [128, KD, T]
    xT = np.ascontiguousarray(xs.T.reshape(KD, 128, T).transpose(1, 0, 2))

    in_maps = [None] * N_CORES
    for q in range(NQ):
        w1q = np.empty((128, E, KQ, KD, 128), ml_dtypes.bfloat16)
        w2q = np.empty((128, E, KD, KQ, 128), ml_dtypes.bfloat16)
        b1q = np.empty((128, E, KQ), np.float32)
        for e in range(E):
            w1e = w1[e][:, q * FQ:(q + 1) * FQ]        # [D, FQ]
            w1q[:, e] = w1e.reshape(KD, 128, KQ, 128).transpose(
                1, 2, 0, 3).astype(ml_dtypes.bfloat16)
            w2e = w2[e][q * FQ:(q + 1) * FQ, :]        # [FQ, D]
            w2q[:, e] = w2e.reshape(KQ, 128, KD, 128).transpose(
                1, 2, 0, 3).astype(ml_dtypes.bfloat16)
            b1q[:, e] = b1[e][q * FQ:(q + 1) * FQ].reshape(KQ, 128).T
        in_maps[q] = {"xT": xT, "w1": w1q, "w2": w2q, "b1": b1q}

    res = bass_utils.run_bass_kernel_spmd(nc, in_maps,
                                          core_ids=list(range(N_CORES)),
                                          trace=TRACE)
    global LAST_RESULT
    LAST_RESULT = res

    acc = res.results[0]["yT"].astype(np.float32)
    for q in range(1, NQ):
        acc += res.results[q]["yT"].astype(np.float32)
    # [128, KD, T] -> [T, D]
    ys = acc.transpose(1, 0, 2).reshape(D, T).T
    out = np.empty((T, D), np.float32)
    out[sort_idx] = ys + b2[eidx[sort_idx]]
    return out.reshape(B, S, D)


# revision 13
# speedup vs baseline: 1.0288x; 1.0288x over previous
"""MoE top-1 routing kernel for Trainium2 (8 NeuronCores).

Reference computation (B=8, S=1024, D=768, E=8, F=3072):
    gates = softmax(x @ gate_w + gate_b); expert_idx = argmax(gates)
    out[t] = gelu(x[t] @ w1[e] + b1[e]) @ w2[e] + b2[e]   for e = expert_idx[t]
    (no gate-probability scaling)

Strategy (v2 — zero-padding feature-parallel):
  * Routing on host in fp64 (softmax is monotonic, so argmax of logits ==
    argmax of gates; observed top-2 logit gaps are >=2e-5, far above fp32
    matmul noise, so this matches the reference's argmax).
  * Every core processes ALL 8192 tokens (sorted by expert) but only a
    1/8 slice of the F dimension of every expert's weights (384 features).
    Core q holds w1[:, q*384:(q+1)*384] and w2[q*384:(q+1)*384, :] for all
    8 experts (9.4 MB bf16) and produces a partial sum of the second
    matmul; the host adds the 8 partials + b2.  This gives exactly T/8
    worth of MACs per core with ZERO padding, for any routing balance —
    vs ~100-token padding for the grouped expert-parallel split.
  * Matmuls in bf16 with fp32 PSUM accumulation; activations stay
    transposed ([feature, token]) so both weight matrices act as the
    stationary matmul operand in their natural layout.  gelu (erf-based)
    on the Scalar engine with the b1 bias fused; FFN2 partial-sums are
    copied PSUM->SBUF as bf16 on the Vector engine and DMA'd out.
  * Head: DMAs are issued in exact consumption order (b1, first expert's
    first w1 chunk, a small 128-token lead tile, ...), batched one-per-
    object so issue cost doesn't serialize; a short PE warmup spins while
    the first ~400 KB lands, then real matmuls start (possibly still at
    the cold 1.2 GHz clock — still useful work) and the HAM clock-gate
    lifts to 2.4 GHz.  Tail: the last tile is kept small and its output
    DMA is split per-chunk so the final write receipt trails as little
    compute as possible.
"""

import sys

try:
    import concourse  # noqa: F401
except ImportError:
    sys.path.insert(0, "/opt/trn_rl_repo")

import numpy as np
import ml_dtypes

import concourse.bass as bass  # noqa: F401
import concourse.tile as tile
import concourse.mybir as mybir
from concourse import bacc
from concourse import bass_utils

BF16 = mybir.dt.bfloat16
F32 = mybir.dt.float32
AF = mybir.ActivationFunctionType

B, S, D, E = 8, 1024, 768, 8
F = 4 * D           # 3072
T = B * S           # 8192
KD = D // 128       # 6 contraction chunks over D
NQ = 8              # F-slice factor (all 8 cores)
FQ = F // NQ        # 384 features per core
KQ = FQ // 128      # 3 chunks over the F-slice
N_CORES = 8
MAX_N = 512         # moving-dim tile (one fp32 PSUM bank)
LEAD = 128          # first tile size (minimal first-matmul DMA dependency)
TAIL = 128          # last tile size (short output-DMA tail)
TAIL2 = 192         # second-to-last tile size
N_WARM = 22         # PE warmup matmuls (~2.4us cold) while head DMAs land

# Debug/profiling knobs (used by the local test harness only).
TRACE = False
LAST_RESULT = None


def _split_tiles(cap):
    """Split a block of `cap` tokens into ceil(cap/512) near-equal tiles."""
    if cap == 0:
        return []
    n = -(-cap // MAX_N)
    base, rem = divmod(cap, n)
    out = []
    off = 0
    for i in range(n):
        sz = base + (1 if i < rem else 0)
        out.append((off, sz))
        off += sz
    return out


def _schedule(counts):
    """(block, tile-offset, width) list over 8 expert blocks, descending
    count order.  First tile is LEAD-sized; last tile is <= TAIL."""
    order = sorted(range(E), key=lambda e: -counts[e])
    offs = np.zeros(E + 1, dtype=int)
    for i, e in enumerate(order):
        offs[i + 1] = offs[i] + counts[e]
    sched = []
    for i, e in enumerate(order):
        tiles = _split_tiles(counts[e])
        if i == 0 and tiles and tiles[0][1] > LEAD + 64:
            o, w = tiles[0]
            tiles = [(o, LEAD), (o + LEAD, w - LEAD)] + tiles[1:]
        if i == E - 1 and tiles and tiles[-1][1] > TAIL + TAIL2 + 64:
            o, w = tiles[-1]
            tiles = tiles[:-1] + [(o, w - TAIL - TAIL2),
                                  (o + w - TAIL - TAIL2, TAIL2),
                                  (o + w - TAIL, TAIL)]
        for (o, w) in tiles:
            sched.append((i, offs[i] + o, w))
    return order, offs, sched


def build_program(counts):
    """Per-core program: 8 expert blocks (descending size) over the full
    sorted token stream; each core computes a 1/8-F partial of FFN1+FFN2."""
    counts = list(counts)
    order, offs, sched = _schedule(counts)
    nc = bacc.Bacc("TRN2", target_bir_lowering=False, debug=False,
                   num_devices=N_CORES)

    xT_d = nc.dram_tensor("xT", (128, KD, T), BF16, kind="ExternalInput")
    w1_d = nc.dram_tensor("w1", (128, E, KQ, KD, 128), BF16,
                          kind="ExternalInput")
    w2_d = nc.dram_tensor("w2", (128, E, KD, KQ, 128), BF16,
                          kind="ExternalInput")
    b1_d = nc.dram_tensor("b1", (128, E, KQ), F32, kind="ExternalInput")
    yT_d = nc.dram_tensor("yT", (128, KD, T), BF16, kind="ExternalOutput")

    with tile.TileContext(nc) as tc:
        with (
            tc.tile_pool(name="wts", bufs=1) as wts,
            tc.tile_pool(name="act", bufs=2) as actp,
            tc.tile_pool(name="yp", bufs=3) as yp,
            tc.tile_pool(name="ps1", bufs=3, space="PSUM") as ps1,
            tc.tile_pool(name="ps2", bufs=5, space="PSUM") as ps2,
        ):
            xT = wts.tile([128, KD, T], BF16, tag="xT")
            w1 = wts.tile([128, E, KQ, KD, 128], BF16, tag="w1")
            w2 = wts.tile([128, E, KD, KQ, 128], BF16, tag="w2")
            b1 = wts.tile([128, E, KQ], F32, tag="b1")
            warm = wts.tile([128, 128], BF16, tag="warm")
            nc.gpsimd.memset(warm[:], 0.0)
            wps = ps2.tile([128, 128], F32, tag="ps2",
                           padded_shape=[128, MAX_N])

            # PE warmup: dummy matmuls keep the PE busy while the head DMAs
            # stream in, flipping the HAM clock gate toward 2.4 GHz before
            # (or shortly after) the real matmul stream starts.
            for _ in range(N_WARM):
                nc.tensor.matmul(wps[:, :], warm[:, :], warm[:, :])

            # DMA plan.  Everything except two tiny head transfers rides
            # the Sync HWDGE queue (a single FIFO gives full control of
            # delivery order); instructions are interleaved with the tile
            # emission so inputs arrive exactly one block ahead of use and
            # output tiles never sit behind a deep input backlog.  The
            # Scalar engine (gelu stream) only issues b1 + the very first
            # w1 chunk up front, and the last tile's output at the end.
            blocks = [i for i in range(E) if counts[order[i]] > 0]
            tiles_of = {i: [(n0, nt) for (bi, n0, nt) in sched if bi == i]
                        for i in blocks}
            e0 = order[blocks[0]]
            nc.scalar.dma_start(b1[:], b1_d[:])
            nc.scalar.dma_start(w1[:, e0, 0], w1_d[:, e0, 0])
            nc.scalar.dma_start(w1[:, e0, 1], w1_d[:, e0, 1])

            def dma_xt(n0, nt):
                nc.sync.dma_start(xT[:, :, n0:n0 + nt],
                                  xT_d[:, :, n0:n0 + nt])

            # Head: block 0 completely, plus the next block's w1.
            b0_tiles = tiles_of[blocks[0]]
            dma_xt(*b0_tiles[0])
            nc.sync.dma_start(w1[:, e0, 2], w1_d[:, e0, 2])
            if len(b0_tiles) > 1:
                dma_xt(*b0_tiles[1])
            nc.sync.dma_start(w2[:, e0], w2_d[:, e0])
            for (n0, nt) in b0_tiles[2:]:
                dma_xt(n0, nt)
            if len(blocks) > 1:
                e1 = order[blocks[1]]
                nc.sync.dma_start(w1[:, e1], w1_d[:, e1])

            # JIT plan: during block k, pull block k+1's w2 + tokens and
            # block k+2's w1, spread round-robin over block k's tiles.
            # jit[tile_global_index] = list of thunks to issue there.
            jit = [[] for _ in sched]
            tile_base = {}
            tb = 0
            for i in blocks:
                tile_base[i] = tb
                tb += len(tiles_of[i])
            for ki, i in enumerate(blocks):
                items = []
                if ki + 1 < len(blocks):
                    nxt = order[blocks[ki + 1]]
                    items.append(lambda e=nxt: nc.sync.dma_start(
                        w2[:, e], w2_d[:, e]))
                    for (n0, nt) in tiles_of[blocks[ki + 1]]:
                        items.append(lambda n0=n0, nt=nt: dma_xt(n0, nt))
                if ki + 2 < len(blocks):
                    nx2 = order[blocks[ki + 2]]
                    items.append(lambda e=nx2: nc.sync.dma_start(
                        w1[:, e], w1_d[:, e]))
                ntiles = len(tiles_of[i])
                for j, th in enumerate(items):
                    jit[tile_base[i] + min(j, ntiles - 1)].append(th)

            def ffn1(e, n0, nt):
                h = actp.tile([128, KQ, nt], BF16, tag="h",
                              padded_shape=[128, KQ, MAX_N])
                for m in range(KQ):
                    ps = ps1.tile([128, nt], F32, tag="ps1",
                                  padded_shape=[128, MAX_N])
                    for k in range(KD):
                        nc.tensor.matmul(
                            ps[:, :],
                            w1[:, e, m, k, :],
                            xT[:, k, n0:n0 + nt],
                            start=(k == 0),
                            stop=(k == KD - 1),
                        )
                    nc.scalar.activation(h[:, m, :], ps[:, :], AF.Gelu,
                                         bias=b1[:, e, m:m + 1])
                return h

            def ffn2(e, n0, nt, h, out_eng):
                y = yp.tile([128, KD, nt], BF16, tag="y",
                            padded_shape=[128, KD, MAX_N])
                for md in range(KD):
                    ps = ps2.tile([128, nt], F32, tag="ps2",
                                  padded_shape=[128, MAX_N])
                    for k in range(KQ):
                        nc.tensor.matmul(
                            ps[:, :],
                            w2[:, e, md, k, :],
                            h[:, k, :],
                            start=(k == 0),
                            stop=(k == KQ - 1),
                        )
                    nc.vector.tensor_copy(y[:, md, :], ps[:, :])
                out_eng.dma_start(yT_d[:, :, n0:n0 + nt], y[:, :, :])

            # Software-pipelined emission: FFN1(t) ahead of FFN2(t-1) so the
            # PE never waits on the gelu of the tile it just produced.  The
            # JIT input DMAs for block k+1 are issued at their scheduled
            # tiles; output DMAs follow each tile's casts on the same Sync
            # queue (the last tile's goes on the empty Scalar queue so its
            # write receipt isn't stuck behind a draining transfer).
            prev = None
            for ti, (b, n0, nt) in enumerate(sched):
                for th in jit[ti]:
                    th()
                h = ffn1(order[b], n0, nt)
                if prev is not None:
                    ffn2(*prev)
                out_eng = nc.scalar if ti >= len(sched) - 2 else nc.sync
                prev = (order[b], n0, nt, h, out_eng)
            if prev is not None:
                ffn2(*prev)

    nc.compile()
    return nc


_PROGRAM_CACHE = {}


def _get_program(counts):
    key = tuple(counts)
    if key not in _PROGRAM_CACHE:
        _PROGRAM_CACHE[key] = build_program(counts)
    return _PROGRAM_CACHE[key]


def kernel(x, gate_w, gate_b, w1, b1, w2, b2):
    x = np.asarray(x)
    w1 = np.asarray(w1)
    b1 = np.asarray(b1)
    w2 = np.asarray(w2)
    b2 = np.asarray(b2)
    xt = x.reshape(T, D)

    # --- Routing on host (fp64; softmax is monotonic => argmax of logits) ---
    logits = xt.astype(np.float64) @ np.asarray(gate_w, np.float64)
    logits += np.asarray(gate_b, np.float64)
    eidx = np.argmax(logits, axis=-1)
    counts = np.bincount(eidx, minlength=E)

    nc = _get_program(counts)
    order, offs, _ = _schedule(counts)

    # Token stream sorted by expert in block order (descending count).
    idx_blocks = [np.nonzero(eidx == e)[0] for e in order]
    sort_idx = np.concatenate(idx_blocks)
    xs = xt[sort_idx].astype(ml_dtypes.bfloat16)        # [T, D]
    # [T, D] -> [128, KD, T]
    xT = np.ascontiguousarray(xs.T.reshape(KD, 128, T).transpose(1, 0, 2))

    in_maps = [None] * N_CORES
    for q in range(NQ):
        w1q = np.empty((128, E, KQ, KD, 128), ml_dtypes.bfloat16)
        w2q = np.empty((128, E, KD, KQ, 128), ml_dtypes.bfloat16)
        b1q = np.empty((128, E, KQ), np.float32)
        for e in range(E):
            w1e = w1[e][:, q * FQ:(q + 1) * FQ]        # [D, FQ]
            w1q[:, e] = w1e.reshape(KD, 128, KQ, 128).transpose(
                1, 2, 0, 3).astype(ml_dtypes.bfloat16)
            w2e = w2[e][q * FQ:(q + 1) * FQ, :]        # [FQ, D]
            w2q[:, e] = w2e.reshape(KQ, 128, KD, 128).transpose(
                1, 2, 0, 3).astype(ml_dtypes.bfloat16)
            b1q[:, e] = b1[e][q * FQ:(q + 1) * FQ].reshape(KQ, 128).T
        in_maps[q] = {"xT": xT, "w1": w1q, "w2": w2q, "b1": b1q}

    res = bass_utils.run_bass_kernel_spmd(nc, in_maps,
                                          core_ids=list(range(N_CORES)),
                                          trace=TRACE)
    global LAST_RESULT
    LAST_RESULT = res

    acc = res.results[0]["yT"].astype(np.float32)
    for q in range(1, NQ):
        acc += res.results[q]["yT"].astype(np.float32)
    # [128, KD, T] -> [T, D]
    ys = acc.transpose(1, 0, 2).reshape(D, T).T
    out = np.empty((T, D), np.float32)
    out[sort_idx] = ys + b2[eidx[sort_idx]]
    return out.reshape(B, S, D)


# revision 14
# speedup vs baseline: 1.0405x; 1.0113x over previous
"""MoE top-1 routing kernel for Trainium2 (8 NeuronCores).

Reference computation (B=8, S=1024, D=768, E=8, F=3072):
    gates = softmax(x @ gate_w + gate_b); expert_idx = argmax(gates)
    out[t] = gelu(x[t] @ w1[e] + b1[e]) @ w2[e] + b2[e]   for e = expert_idx[t]
    (no gate-probability scaling)

Strategy (v2 — zero-padding feature-parallel):
  * Routing on host in fp64 (softmax is monotonic, so argmax of logits ==
    argmax of gates; observed top-2 logit gaps are >=2e-5, far above fp32
    matmul noise, so this matches the reference's argmax).
  * Every core processes ALL 8192 tokens (sorted by expert) but only a
    1/8 slice of the F dimension of every expert's weights (384 features).
    Core q holds w1[:, q*384:(q+1)*384] and w2[q*384:(q+1)*384, :] for all
    8 experts (9.4 MB bf16) and produces a partial sum of the second
    matmul; the host adds the 8 partials + b2.  This gives exactly T/8
    worth of MACs per core with ZERO padding, for any routing balance —
    vs ~100-token padding for the grouped expert-parallel split.
  * Matmuls in bf16 with fp32 PSUM accumulation; activations stay
    transposed ([feature, token]) so both weight matrices act as the
    stationary matmul operand in their natural layout.  gelu (erf-based)
    on the Scalar engine with the b1 bias fused; FFN2 partial-sums are
    copied PSUM->SBUF as bf16 on the Vector engine and DMA'd out.
  * Head: DMAs are issued in exact consumption order (b1, first expert's
    first w1 chunk, a small 128-token lead tile, ...), batched one-per-
    object so issue cost doesn't serialize; a short PE warmup spins while
    the first ~400 KB lands, then real matmuls start (possibly still at
    the cold 1.2 GHz clock — still useful work) and the HAM clock-gate
    lifts to 2.4 GHz.  Tail: the last tile is kept small and its output
    DMA is split per-chunk so the final write receipt trails as little
    compute as possible.
"""

import sys

try:
    import concourse  # noqa: F401
except ImportError:
    sys.path.insert(0, "/opt/trn_rl_repo")

import numpy as np
import ml_dtypes

import concourse.bass as bass  # noqa: F401
import concourse.tile as tile
import concourse.mybir as mybir
from concourse import bacc
from concourse import bass_utils

BF16 = mybir.dt.bfloat16
F32 = mybir.dt.float32
AF = mybir.ActivationFunctionType

B, S, D, E = 8, 1024, 768, 8
F = 4 * D           # 3072
T = B * S           # 8192
KD = D // 128       # 6 contraction chunks over D
NQ = 8              # F-slice factor (all 8 cores)
FQ = F // NQ        # 384 features per core
KQ = FQ // 128      # 3 chunks over the F-slice
N_CORES = 8
MAX_N = 512         # moving-dim tile (one fp32 PSUM bank)
LEAD = 128          # first tile size (minimal first-matmul DMA dependency)
TAIL = 128          # last tile size (short output-DMA tail)
TAIL2 = 192         # second-to-last tile size
N_WARM = 34         # PE warmup matmuls (~2.4us cold) while head DMAs land

# Debug/profiling knobs (used by the local test harness only).
TRACE = False
LAST_RESULT = None


def _split_tiles(cap):
    """Split a block of `cap` tokens into ceil(cap/512) near-equal tiles."""
    if cap == 0:
        return []
    n = -(-cap // MAX_N)
    base, rem = divmod(cap, n)
    out = []
    off = 0
    for i in range(n):
        sz = base + (1 if i < rem else 0)
        out.append((off, sz))
        off += sz
    return out


def _schedule(counts):
    """(block, tile-offset, width) list over 8 expert blocks, descending
    count order.  First tile is LEAD-sized; last tile is <= TAIL."""
    order = sorted(range(E), key=lambda e: -counts[e])
    offs = np.zeros(E + 1, dtype=int)
    for i, e in enumerate(order):
        offs[i + 1] = offs[i] + counts[e]
    sched = []
    for i, e in enumerate(order):
        tiles = _split_tiles(counts[e])
        if i == 0 and tiles and tiles[0][1] > LEAD + 64:
            o, w = tiles[0]
            tiles = [(o, LEAD), (o + LEAD, w - LEAD)] + tiles[1:]
        if i == E - 1 and tiles and tiles[-1][1] > TAIL + TAIL2 + 64:
            o, w = tiles[-1]
            tiles = tiles[:-1] + [(o, w - TAIL - TAIL2),
                                  (o + w - TAIL - TAIL2, TAIL2),
                                  (o + w - TAIL, TAIL)]
        for (o, w) in tiles:
            sched.append((i, offs[i] + o, w))
    return order, offs, sched


def build_program(counts):
    """Per-core program: 8 expert blocks (descending size) over the full
    sorted token stream; each core computes a 1/8-F partial of FFN1+FFN2."""
    counts = list(counts)
    order, offs, sched = _schedule(counts)
    nc = bacc.Bacc("TRN2", target_bir_lowering=False, debug=False,
                   num_devices=N_CORES)

    xT_d = nc.dram_tensor("xT", (128, KD, T), BF16, kind="ExternalInput")
    w1_d = nc.dram_tensor("w1", (128, E, KQ, KD, 128), BF16,
                          kind="ExternalInput")
    w2_d = nc.dram_tensor("w2", (128, E, KD, KQ, 128), BF16,
                          kind="ExternalInput")
    b1_d = nc.dram_tensor("b1", (128, E, KQ), F32, kind="ExternalInput")
    yT_d = nc.dram_tensor("yT", (128, KD, T), BF16, kind="ExternalOutput")

    with tile.TileContext(nc) as tc:
        with (
            tc.tile_pool(name="wts", bufs=1) as wts,
            tc.tile_pool(name="act", bufs=2) as actp,
            tc.tile_pool(name="yp", bufs=4) as yp,
            tc.tile_pool(name="ps1", bufs=3, space="PSUM") as ps1,
            tc.tile_pool(name="ps2", bufs=5, space="PSUM") as ps2,
        ):
            xT = wts.tile([128, KD, T], BF16, tag="xT")
            w1 = wts.tile([128, E, KQ, KD, 128], BF16, tag="w1")
            w2 = wts.tile([128, E, KD, KQ, 128], BF16, tag="w2")
            b1 = wts.tile([128, E, KQ], F32, tag="b1")
            warm = wts.tile([128, 128], BF16, tag="warm")
            nc.gpsimd.memset(warm[:], 0.0)
            wps = ps2.tile([128, 128], F32, tag="ps2",
                           padded_shape=[128, MAX_N])

            # PE warmup: dummy matmuls keep the PE busy while the head DMAs
            # stream in, flipping the HAM clock gate toward 2.4 GHz before
            # (or shortly after) the real matmul stream starts.
            for _ in range(N_WARM):
                nc.tensor.matmul(wps[:, :], warm[:, :], warm[:, :])

            # DMA plan.  Everything except two tiny head transfers rides
            # the Sync HWDGE queue (a single FIFO gives full control of
            # delivery order); instructions are interleaved with the tile
            # emission so inputs arrive exactly one block ahead of use and
            # output tiles never sit behind a deep input backlog.  The
            # Scalar engine (gelu stream) only issues b1 + the very first
            # w1 chunk up front, and the last tile's output at the end.
            blocks = [i for i in range(E) if counts[order[i]] > 0]
            tiles_of = {i: [(n0, nt) for (bi, n0, nt) in sched if bi == i]
                        for i in blocks}
            e0 = order[blocks[0]]
            nc.scalar.dma_start(b1[:], b1_d[:])
            nc.scalar.dma_start(w1[:, e0, 0], w1_d[:, e0, 0])
            nc.scalar.dma_start(w1[:, e0, 1], w1_d[:, e0, 1])

            def dma_xt(n0, nt):
                nc.sync.dma_start(xT[:, :, n0:n0 + nt],
                                  xT_d[:, :, n0:n0 + nt])

            # Head: block 0 completely, plus the next block's w1.
            b0_tiles = tiles_of[blocks[0]]
            dma_xt(*b0_tiles[0])
            nc.sync.dma_start(w1[:, e0, 2], w1_d[:, e0, 2])
            if len(b0_tiles) > 1:
                dma_xt(*b0_tiles[1])
            nc.sync.dma_start(w2[:, e0], w2_d[:, e0])
            for (n0, nt) in b0_tiles[2:]:
                dma_xt(n0, nt)
            if len(blocks) > 1:
                e1 = order[blocks[1]]
                nc.sync.dma_start(w1[:, e1], w1_d[:, e1])

            # JIT plan: during block k, pull block k+1's w2 + tokens and
            # block k+2's w1, spread round-robin over block k's tiles.
            # jit[tile_global_index] = list of thunks to issue there.
            jit = [[] for _ in sched]
            tile_base = {}
            tb = 0
            for i in blocks:
                tile_base[i] = tb
                tb += len(tiles_of[i])
            for ki, i in enumerate(blocks):
                items = []
                if ki + 1 < len(blocks):
                    nxt = order[blocks[ki + 1]]
                    items.append(lambda e=nxt: nc.sync.dma_start(
                        w2[:, e], w2_d[:, e]))
                    for (n0, nt) in tiles_of[blocks[ki + 1]]:
                        items.append(lambda n0=n0, nt=nt: dma_xt(n0, nt))
                if ki + 2 < len(blocks):
                    nx2 = order[blocks[ki + 2]]
                    items.append(lambda e=nx2: nc.sync.dma_start(
                        w1[:, e], w1_d[:, e]))
                ntiles = len(tiles_of[i])
                for j, th in enumerate(items):
                    jit[tile_base[i] + min(j, ntiles - 1)].append(th)

            def ffn1(e, n0, nt):
                h = actp.tile([128, KQ, nt], BF16, tag="h",
                              padded_shape=[128, KQ, MAX_N])
                for m in range(KQ):
                    ps = ps1.tile([128, nt], F32, tag="ps1",
                                  padded_shape=[128, MAX_N])
                    for k in range(KD):
                        nc.tensor.matmul(
                            ps[:, :],
                            w1[:, e, m, k, :],
                            xT[:, k, n0:n0 + nt],
                            start=(k == 0),
                            stop=(k == KD - 1),
                        )
                    nc.scalar.activation(h[:, m, :], ps[:, :], AF.Gelu,
                                         bias=b1[:, e, m:m + 1])
                return h

            def ffn2(e, n0, nt, h, out_eng):
                y = yp.tile([128, KD, nt], BF16, tag="y",
                            padded_shape=[128, KD, MAX_N])
                for md in range(KD):
                    ps = ps2.tile([128, nt], F32, tag="ps2",
                                  padded_shape=[128, MAX_N])
                    for k in range(KQ):
                        nc.tensor.matmul(
                            ps[:, :],
                            w2[:, e, md, k, :],
                            h[:, k, :],
                            start=(k == 0),
                            stop=(k == KQ - 1),
                        )
                    nc.vector.tensor_copy(y[:, md, :], ps[:, :])
                out_eng.dma_start(yT_d[:, :, n0:n0 + nt], y[:, :, :])

            # Software-pipelined emission: FFN1(t) ahead of FFN2(t-1) so the
            # PE never waits on the gelu of the tile it just produced.  The
            # JIT input DMAs for block k+1 are issued at their scheduled
            # tiles; output DMAs follow each tile's casts on the same Sync
            # queue (the last tile's goes on the empty Scalar queue so its
            # write receipt isn't stuck behind a draining transfer).
            prev = None
            for ti, (b, n0, nt) in enumerate(sched):
                for th in jit[ti]:
                    th()
                h = ffn1(order[b], n0, nt)
                if prev is not None:
                    ffn2(*prev)
                n_left = len(sched) - 1 - ti
                out_eng = nc.scalar if (n_left < 4 and n_left % 2 == 0) \
                    else nc.sync
                prev = (order[b], n0, nt, h, out_eng)
            if prev is not None:
                ffn2(*prev)

    nc.compile()
    return nc


_PROGRAM_CACHE = {}


def _get_program(counts):
    key = tuple(counts)
    if key not in _PROGRAM_CACHE:
        _PROGRAM_CACHE[key] = build_program(counts)
    return _PROGRAM_CACHE[key]


def kernel(x, gate_w, gate_b, w1, b1, w2, b2):
    x = np.asarray(x)
    w1 = np.asarray(w1)
    b1 = np.asarray(b1)
    w2 = np.asarray(w2)
    b2 = np.asarray(b2)
    xt = x.reshape(T, D)

    # --- Routing on host (fp64; softmax is monotonic => argmax of logits) ---
    logits = xt.astype(np.float64) @ np.asarray(gate_w, np.float64)
    logits += np.asarray(gate_b, np.float64)
    eidx = np.argmax(logits, axis=-1)
    counts = np.bincount(eidx, minlength=E)

    nc = _get_program(counts)
    order, offs, _ = _schedule(counts)

    # Token stream sorted by expert in block order (descending count).
    idx_blocks = [np.nonzero(eidx == e)[0] for e in order]
    sort_idx = np.concatenate(idx_blocks)
    xs = xt[sort_idx].astype(ml_dtypes.bfloat16)        # [T, D]
    # [T, D] -> [128, KD, T]
    xT = np.ascontiguousarray(xs.T.reshape(KD, 128, T).transpose(1, 0, 2))

    in_maps = [None] * N_CORES
    for q in range(NQ):
        w1q = np.empty((128, E, KQ, KD, 128), ml_dtypes.bfloat16)
        w2q = np.empty((128, E, KD, KQ, 128), ml_dtypes.bfloat16)
        b1q = np.empty((128, E, KQ), np.float32)
        for e in range(E):
            w1e = w1[e][:, q * FQ:(q + 1) * FQ]        # [D, FQ]
            w1q[:, e] = w1e.reshape(KD, 128, KQ, 128).transpose(
                1, 2, 0, 3).astype(ml_dtypes.bfloat16)
            w2e = w2[e][q * FQ:(q + 1) * FQ, :]        # [FQ, D]
            w2q[:, e] = w2e.reshape(KQ, 128, KD, 128).transpose(
                1, 2, 0, 3).astype(ml_dtypes.bfloat16)
            b1q[:, e] = b1[e][q * FQ:(q + 1) * FQ].reshape(KQ, 128).T
        in_maps[q] = {"xT": xT, "w1": w1q, "w2": w2q, "b1": b1q}

    res = bass_utils.run_bass_kernel_spmd(nc, in_maps,
                                          core_ids=list(range(N_CORES)),
                                          trace=TRACE)
    global LAST_RESULT
    LAST_RESULT = res

    acc = res.results[0]["yT"].astype(np.float32)
    for q in range(1, NQ):
        acc += res.results[q]["yT"].astype(np.float32)
    # [128, KD, T] -> [T, D]
    ys = acc.transpose(1, 0, 2).reshape(D, T).T
    out = np.empty((T, D), np.float32)
    out[sort_idx] = ys + b2[eidx[sort_idx]]
    return out.reshape(B, S, D)


# revision 15
# speedup vs baseline: 1.0484x; 1.0076x over previous
"""MoE top-1 routing kernel for Trainium2 (8 NeuronCores).

Reference computation (B=8, S=1024, D=768, E=8, F=3072):
    gates = softmax(x @ gate_w + gate_b); expert_idx = argmax(gates)
    out[t] = gelu(x[t] @ w1[e] + b1[e]) @ w2[e] + b2[e]   for e = expert_idx[t]
    (no gate-probability scaling)

Strategy (v2 — zero-padding feature-parallel):
  * Routing on host in fp64 (softmax is monotonic, so argmax of logits ==
    argmax of gates; observed top-2 logit gaps are >=2e-5, far above fp32
    matmul noise, so this matches the reference's argmax).
  * Every core processes ALL 8192 tokens (sorted by expert) but only a
    1/8 slice of the F dimension of every expert's weights (384 features).
    Core q holds w1[:, q*384:(q+1)*384] and w2[q*384:(q+1)*384, :] for all
    8 experts (9.4 MB bf16) and produces a partial sum of the second
    matmul; the host adds the 8 partials + b2.  This gives exactly T/8
    worth of MACs per core with ZERO padding, for any routing balance —
    vs ~100-token padding for the grouped expert-parallel split.
  * Matmuls in bf16 with fp32 PSUM accumulation; activations stay
    transposed ([feature, token]) so both weight matrices act as the
    stationary matmul operand in their natural layout.  gelu (erf-based)
    on the Scalar engine with the b1 bias fused; FFN2 partial-sums are
    copied PSUM->SBUF as bf16 on the Vector engine and DMA'd out.
  * Head: DMAs are issued in exact consumption order (b1, first expert's
    first w1 chunk, a small 128-token lead tile, ...), batched one-per-
    object so issue cost doesn't serialize; a short PE warmup spins while
    the first ~400 KB lands, then real matmuls start (possibly still at
    the cold 1.2 GHz clock — still useful work) and the HAM clock-gate
    lifts to 2.4 GHz.  Tail: the last tile is kept small and its output
    DMA is split per-chunk so the final write receipt trails as little
    compute as possible.
"""

import sys

try:
    import concourse  # noqa: F401
except ImportError:
    sys.path.insert(0, "/opt/trn_rl_repo")

import numpy as np
import ml_dtypes

import concourse.bass as bass  # noqa: F401
import concourse.tile as tile
import concourse.mybir as mybir
from concourse import bacc
from concourse import bass_utils

BF16 = mybir.dt.bfloat16
F32 = mybir.dt.float32
AF = mybir.ActivationFunctionType

B, S, D, E = 8, 1024, 768, 8
F = 4 * D           # 3072
T = B * S           # 8192
KD = D // 128       # 6 contraction chunks over D
NQ = 8              # F-slice factor (all 8 cores)
FQ = F // NQ        # 384 features per core
KQ = FQ // 128      # 3 chunks over the F-slice
N_CORES = 8
MAX_N = 512         # moving-dim tile (one fp32 PSUM bank)
LEAD = 128          # first tile size (minimal first-matmul DMA dependency)
TAIL = 128          # last tile size (short output-DMA tail)
TAIL2 = 192         # second-to-last tile size
N_WARM = 34         # PE warmup matmuls (~2.4us cold) while head DMAs land

# Debug/profiling knobs (used by the local test harness only).
TRACE = False
LAST_RESULT = None


def _split_tiles(cap):
    """Split a block of `cap` tokens into ceil(cap/512) near-equal tiles."""
    if cap == 0:
        return []
    n = -(-cap // MAX_N)
    base, rem = divmod(cap, n)
    out = []
    off = 0
    for i in range(n):
        sz = base + (1 if i < rem else 0)
        out.append((off, sz))
        off += sz
    return out


def _schedule(counts):
    """(block, tile-offset, width) list over 8 expert blocks, descending
    count order.  First tile is LEAD-sized; last tile is <= TAIL."""
    order = sorted(range(E), key=lambda e: -counts[e])
    offs = np.zeros(E + 1, dtype=int)
    for i, e in enumerate(order):
        offs[i + 1] = offs[i] + counts[e]
    sched = []
    for i, e in enumerate(order):
        tiles = _split_tiles(counts[e])
        if i == 0 and tiles and tiles[0][1] > LEAD + 64:
            o, w = tiles[0]
            tiles = [(o, LEAD), (o + LEAD, w - LEAD)] + tiles[1:]
        if i == E - 1 and tiles and tiles[-1][1] > TAIL + TAIL2 + 64:
            o, w = tiles[-1]
            tiles = tiles[:-1] + [(o, w - TAIL - TAIL2),
                                  (o + w - TAIL - TAIL2, TAIL2),
                                  (o + w - TAIL, TAIL)]
        for (o, w) in tiles:
            sched.append((i, offs[i] + o, w))
    return order, offs, sched


def build_program(counts):
    """Per-core program: 8 expert blocks (descending size) over the full
    sorted token stream; each core computes a 1/8-F partial of FFN1+FFN2."""
    counts = list(counts)
    order, offs, sched = _schedule(counts)
    nc = bacc.Bacc("TRN2", target_bir_lowering=False, debug=False,
                   num_devices=N_CORES)

    xT_d = nc.dram_tensor("xT", (128, KD, T), BF16, kind="ExternalInput")
    w1_d = nc.dram_tensor("w1", (128, E, KQ, KD, 128), BF16,
                          kind="ExternalInput")
    w2_d = nc.dram_tensor("w2", (128, E, KD, KQ, 128), BF16,
                          kind="ExternalInput")
    b1_d = nc.dram_tensor("b1", (128, E, KQ), F32, kind="ExternalInput")
    yT_d = nc.dram_tensor("yT", (128, KD, T), BF16, kind="ExternalOutput")

    with tile.TileContext(nc) as tc:
        with (
            tc.tile_pool(name="wts", bufs=1) as wts,
            tc.tile_pool(name="act", bufs=2) as actp,
            tc.tile_pool(name="yp", bufs=4) as yp,
            tc.tile_pool(name="ps1", bufs=3, space="PSUM") as ps1,
            tc.tile_pool(name="ps2", bufs=5, space="PSUM") as ps2,
        ):
            xT = wts.tile([128, KD, T], BF16, tag="xT")
            w1 = wts.tile([128, E, KQ, KD, 128], BF16, tag="w1")
            w2 = wts.tile([128, E, KD, KQ, 128], BF16, tag="w2")
            b1 = wts.tile([128, E, KQ], F32, tag="b1")
            warm = wts.tile([128, 128], BF16, tag="warm")
            nc.gpsimd.memset(warm[:], 0.0)
            wps = ps2.tile([128, 128], F32, tag="ps2",
                           padded_shape=[128, MAX_N])

            # PE warmup: dummy matmuls keep the PE busy while the head DMAs
            # stream in, flipping the HAM clock gate toward 2.4 GHz before
            # (or shortly after) the real matmul stream starts.
            for _ in range(N_WARM):
                nc.tensor.matmul(wps[:, :], warm[:, :], warm[:, :])

            # DMA plan.  Everything except two tiny head transfers rides
            # the Sync HWDGE queue (a single FIFO gives full control of
            # delivery order); instructions are interleaved with the tile
            # emission so inputs arrive exactly one block ahead of use and
            # output tiles never sit behind a deep input backlog.  The
            # Scalar engine (gelu stream) only issues b1 + the very first
            # w1 chunk up front, and the last tile's output at the end.
            blocks = [i for i in range(E) if counts[order[i]] > 0]
            tiles_of = {i: [(n0, nt) for (bi, n0, nt) in sched if bi == i]
                        for i in blocks}
            e0 = order[blocks[0]]
            nc.scalar.dma_start(b1[:], b1_d[:])
            nc.scalar.dma_start(w1[:, e0, 0], w1_d[:, e0, 0])
            nc.scalar.dma_start(w1[:, e0, 1], w1_d[:, e0, 1])

            def dma_xt(n0, nt):
                nc.sync.dma_start(xT[:, :, n0:n0 + nt],
                                  xT_d[:, :, n0:n0 + nt])

            # Head: block 0 completely, plus the next block's w1.
            b0_tiles = tiles_of[blocks[0]]
            dma_xt(*b0_tiles[0])
            nc.sync.dma_start(w1[:, e0, 2], w1_d[:, e0, 2])
            if len(b0_tiles) > 1:
                dma_xt(*b0_tiles[1])
            nc.sync.dma_start(w2[:, e0], w2_d[:, e0])
            for (n0, nt) in b0_tiles[2:]:
                dma_xt(n0, nt)
            if len(blocks) > 1:
                e1 = order[blocks[1]]
                nc.sync.dma_start(w1[:, e1], w1_d[:, e1])

            # JIT plan: during block k, pull block k+1's w2 + tokens and
            # block k+2's w1, spread round-robin over block k's tiles.
            # jit[tile_global_index] = list of thunks to issue there.
            jit = [[] for _ in sched]
            tile_base = {}
            tb = 0
            for i in blocks:
                tile_base[i] = tb
                tb += len(tiles_of[i])
            for ki, i in enumerate(blocks):
                items = []
                if ki + 1 < len(blocks):
                    nxt = order[blocks[ki + 1]]
                    items.append(lambda e=nxt: nc.sync.dma_start(
                        w2[:, e], w2_d[:, e]))
                    for (n0, nt) in tiles_of[blocks[ki + 1]]:
                        items.append(lambda n0=n0, nt=nt: dma_xt(n0, nt))
                if ki + 2 < len(blocks):
                    nx2 = order[blocks[ki + 2]]
                    items.append(lambda e=nx2: nc.sync.dma_start(
                        w1[:, e], w1_d[:, e]))
                ntiles = len(tiles_of[i])
                for j, th in enumerate(items):
                    jit[tile_base[i] + min(j, ntiles - 1)].append(th)

            def ffn1(e, n0, nt):
                h = actp.tile([128, KQ, nt], BF16, tag="h",
                              padded_shape=[128, KQ, MAX_N])
                for m in range(KQ):
                    ps = ps1.tile([128, nt], F32, tag="ps1",
                                  padded_shape=[128, MAX_N])
                    for k in range(KD):
                        nc.tensor.matmul(
                            ps[:, :],
                            w1[:, e, m, k, :],
                            xT[:, k, n0:n0 + nt],
                            start=(k == 0),
                            stop=(k == KD - 1),
                        )
                    nc.scalar.activation(h[:, m, :], ps[:, :], AF.Gelu,
                                         bias=b1[:, e, m:m + 1])
                return h

            def ffn2(e, n0, nt, h, out_eng):
                y = yp.tile([128, KD, nt], BF16, tag="y",
                            padded_shape=[128, KD, MAX_N])
                for md in range(KD):
                    ps = ps2.tile([128, nt], F32, tag="ps2",
                                  padded_shape=[128, MAX_N])
                    for k in range(KQ):
                        nc.tensor.matmul(
                            ps[:, :],
                            w2[:, e, md, k, :],
                            h[:, k, :],
                            start=(k == 0),
                            stop=(k == KQ - 1),
                        )
                    nc.vector.tensor_copy(y[:, md, :], ps[:, :])
                out_eng.dma_start(yT_d[:, :, n0:n0 + nt], y[:, :, :])

            # Software-pipelined emission: FFN1(t) ahead of FFN2(t-1) so the
            # PE never waits on the gelu of the tile it just produced.  The
            # JIT input DMAs for block k+1 are issued at their scheduled
            # tiles; output DMAs follow each tile's casts on the same Sync
            # queue (the last tile's goes on the empty Scalar queue so its
            # write receipt isn't stuck behind a draining transfer).
            prev = None
            for ti, (b, n0, nt) in enumerate(sched):
                for th in jit[ti]:
                    th()
                h = ffn1(order[b], n0, nt)
                if prev is not None:
                    ffn2(*prev)
                n_left = len(sched) - 1 - ti
                out_eng = nc.scalar if n_left < 3 else nc.sync
                prev = (order[b], n0, nt, h, out_eng)
            if prev is not None:
                ffn2(*prev)

    nc.compile()
    return nc


_PROGRAM_CACHE = {}


def _get_program(counts):
    key = tuple(counts)
    if key not in _PROGRAM_CACHE:
        _PROGRAM_CACHE[key] = build_program(counts)
    return _PROGRAM_CACHE[key]


def kernel(x, gate_w, gate_b, w1, b1, w2, b2):
    x = np.asarray(x)
    w1 = np.asarray(w1)
    b1 = np.asarray(b1)
    w2 = np.asarray(w2)
    b2 = np.asarray(b2)
    xt = x.reshape(T, D)

    # --- Routing on host (fp64; softmax is monotonic => argmax of logits) ---
    logits = xt.astype(np.float64) @ np.asarray(gate_w, np.float64)
    logits += np.asarray(gate_b, np.float64)
    eidx = np.argmax(logits, axis=-1)
    counts = np.bincount(eidx, minlength=E)

    nc = _get_program(counts)
    order, offs, _ = _schedule(counts)

    # Token stream sorted by expert in block order (descending count).
    idx_blocks = [np.nonzero(eidx == e)[0] for e in order]
    sort_idx = np.concatenate(idx_blocks)
    xs = xt[sort_idx].astype(ml_dtypes.bfloat16)        # [T, D]
    # [T, D] -> [128, KD, T]
    xT = np.ascontiguousarray(xs.T.reshape(KD, 128, T).transpose(1, 0, 2))

    in_maps = [None] * N_CORES
    for q in range(NQ):
        w1q = np.empty((128, E, KQ, KD, 128), ml_dtypes.bfloat16)
        w2q = np.empty((128, E, KD, KQ, 128), ml_dtypes.bfloat16)
        b1q = np.empty((128, E, KQ), np.float32)
        for e in range(E):
            w1e = w1[e][:, q * FQ:(q + 1) * FQ]        # [D, FQ]
            w1q[:, e] = w1e.reshape(KD, 128, KQ, 128).transpose(
                1, 2, 0, 3).astype(ml_dtypes.bfloat16)
            w2e = w2[e][q * FQ:(q + 1) * FQ, :]        # [FQ, D]
            w2q[:, e] = w2e.reshape(KQ, 128, KD, 128).transpose(
                1, 2, 0, 3).astype(ml_dtypes.bfloat16)
            b1q[:, e] = b1[e][q * FQ:(q + 1) * FQ].reshape(KQ, 128).T
        in_maps[q] = {"xT": xT, "w1": w1q, "w2": w2q, "b1": b1q}

    res = bass_utils.run_bass_kernel_spmd(nc, in_maps,
                                          core_ids=list(range(N_CORES)),
                                          trace=TRACE)
    global LAST_RESULT
    LAST_RESULT = res

    acc = res.results[0]["yT"].astype(np.float32)
    for q in range(1, NQ):
        acc += res.results[q]["yT"].astype(np.float32)
    # [128, KD, T] -> [T, D]
    ys = acc.transpose(1, 0, 2).reshape(D, T).T
    out = np.empty((T, D), np.float32)
    out[sort_idx] = ys + b2[eidx[sort_idx]]
    return out.reshape(B, S, D)
